# revision 19
# baseline (speedup 1.0000x reference)
"""Trainium2 Bass kernel for nn_Attn_head_9543417332154 (GNN attention head).

Reference computation (B=1, N=8192, C=256, O=64):
    sf[j, o]  = x[j] @ W1.T + b1                    # per-node linear
    f1[i] = sf[i] @ a1 + ba1 ; f2[j] = sf[j] @ a2 + ba2
    logits[i, j] = leaky_relu(f1[i] + f2[j], 0.01)
    coefs = softmax(logits, axis=0 over i)          # nn.Softmax(dim=1)
    ret[i, o] = sum_j coefs[i, j] * sf[j, o] ; out = elu(ret)

Device algorithm (no N x N matrix in HBM, single tiny collective):
  With s = f1[i] + f2[j] and mask = (s >= 0):
    exp(lrelu(s)) = mask * A1[i]B1[j] + (1 - mask) * A2[i]B2[j]
    A1 = exp(f1), B1 = exp(f2), A2 = exp(.01 f1), B2 = exp(.01 f2)
  Every core redundantly computes sf/f1/f2 for ALL nodes from the full x
  (cheap GEMM; removes two collectives and all strided cross-core gathers).
  f1/f2 kept bias-free on chip; the scalar biases c1 = b1@a1+ba1 and
  c2 = b1@a2+ba2 are folded into the exp biases and mask thresholds.
  Pass 1 (own j-shard, all i) is INTERLEAVED with the sf GEMM per 4-block
  group to keep the PE dense, psum convention +-1/2:
    m[j] = sum_i (mask - 1/2) A[i]  =  M[j] - SA/2   (M = masked sum, SA = full)
  ONE AllGather of m rows (bf16) [2, 1024] -> [16, 1024]; PE-transpose to cols;
    D[j] = B1 M1 + B2 (SA2 - M2);  scBk[j] = Bk[j] / D[j]   (all j, every core)
  Pass 2 (own i-shard, all j), same +-1/2 mask trick:
    Qk[j, o] = scBk[j] * sf[j, o]
    psum_ret = sum_j (mask - 1/2) Q ; psum_t = sum_j Q/2
    R1 = psum_ret[0:64] + T1/2 ; R2 - T2 = psum_ret[64:128] - T2/2
    ret[i, o] = A1[i] R1 - A2[i] (R2 - T2);  out = elu(ret)

Sharding: output rows (own i) sharded 1024/core; everything else replicated.
"""
import functools
import numpy as np

import concourse.bass as bass
import concourse.bacc as bacc
import concourse.tile as tile
import concourse.mybir as mybir
from concourse.bass_utils import run_bass_kernel_spmd
from concourse.masks import make_identity

F32 = mybir.dt.float32
BF16 = mybir.dt.bfloat16
AF = mybir.ActivationFunctionType
OP = mybir.AluOpType

NCORES = 8
N = 8192
C = 256
O = 64
NS = N // NCORES           # 1024 rows per core
T = NS // 128              # 8 local 128-row tiles
NT = N // 128              # 64 global 128-row tiles
NQ = 2                     # x load column chunks per c-half (1 MB each)


def build_kernel():
    nc = bacc.Bacc(None, target_bir_lowering=False)

    xT = nc.dram_tensor("xT", [C, N], BF16, kind="ExternalInput")       # full x^T
    xoT = nc.dram_tensor("xoT", [C, NS], BF16, kind="ExternalInput")    # own shard x^T
    mov66 = nc.dram_tensor("mov66", [C, 66], BF16, kind="ExternalInput")
    bias66 = nc.dram_tensor("bias66", [1, 66], F32, kind="ExternalInput")
    out = nc.dram_tensor("out", [O, NS], F32, kind="ExternalOutput")    # ret^T shard

    with tile.TileContext(nc) as tc:
        with (
            tc.tile_pool(name="const", bufs=1) as cpool,
            tc.tile_pool(name="work", bufs=3) as wpool,
            tc.tile_pool(name="rows", bufs=1) as rpool,
            tc.tile_pool(name="mask", bufs=4) as mpool,
            tc.tile_pool(name="psA", bufs=2, space="PSUM") as psA,
            tc.tile_pool(name="psM", bufs=1, space="PSUM") as psM,
            tc.tile_pool(name="psT", bufs=1, space="PSUM") as psT,
            tc.tile_pool(name="dram", bufs=1, space="DRAM") as dram,
        ):
            # ---- DRAM buffers ----
            ag0_in = dram.tile([1, 1], F32)
            ag0_out = dram.tile([NCORES, 1], F32, addr_space="Shared")
            fownd = dram.tile([2, NS], BF16)      # raw f1own/f2own rows
            fownd1 = dram.tile([1, NS], F32)      # raw f1own row f32
            dsa = dram.tile([2, 1], F32)          # SA1, SA2
            ag_in = dram.tile([2, NS], BF16)      # m rows (M - SA/2)
            ag_out = dram.tile([2 * NCORES, NS], BF16, addr_space="Shared")

            # ---- constants ----
            ones_col = cpool.tile([128, 1], BF16)
            nc.gpsimd.memset(ones_col[:], 1.0)
            halfs_col = cpool.tile([128, 1], BF16)
            nc.gpsimd.memset(halfs_col[:], 0.5)
            ident16 = cpool.tile([16, 16], BF16)
            make_identity(nc, ident16[:])

            # dummy collective: absorbs CC-stream warmup + first-op latency
            zz = cpool.tile([1, 1], F32)
            nc.gpsimd.memset(zz[:], 0.0)
            nc.sync.dma_start(ag0_in[:], zz[:])
            nc.gpsimd.collective_compute(
                "AllGather", OP.bypass, ins=[ag0_in[:]], outs=[ag0_out[:]],
                replica_groups=[list(range(NCORES))],
            )

            movs = cpool.tile([128, 2, 66], BF16)
            for c in range(2):
                nc.scalar.dma_start(movs[:, c, :], mov66[c * 128:(c + 1) * 128, :])
            bias_bc = cpool.tile([128, 66], F32)
            nc.scalar.dma_start(bias_bc[:], bias66[0:1, :].partition_broadcast(128))
            c1b = bias_bc[:, 64:65]
            c2b = bias_bc[:, 65:66]
            css = cpool.tile([128, 2], F32)       # 0.01*c1, 0.01*c2
            nc.vector.tensor_scalar(css[:], bias_bc[:, 64:66], 0.01, None, OP.mult)
            ccb = cpool.tile([128, 1], F32)       # c1 + c2
            nc.vector.tensor_tensor(ccb[:], c1b, c2b, OP.add)

            # ---- phase A0: own-shard raw f1/f2 rows (for mask broadcasts) ----
            xo = cpool.tile([128, 2, NS], BF16)
            for c in range(2):
                nc.scalar.dma_start(xo[:, c, :], xoT[c * 128:(c + 1) * 128, :])
            psF = psM.tile([2, NS], F32, tag="m", bufs=1)  # same slot as psum_m
            for h in range(2):
                sl = slice(h * 512, (h + 1) * 512)
                for c in range(2):
                    nc.tensor.matmul(
                        psF[:, sl], movs[:, c, 64:66], xo[:, c, sl],
                        start=(c == 0), stop=(c == 1),
                    )
            fown_b = rpool.tile([2, NS], BF16)
            nc.vector.tensor_copy(fown_b[:], psF[:])
            fown_1 = rpool.tile([1, NS], F32)
            nc.scalar.activation(fown_1[:], psF[0:1, :], AF.Copy)
            nc.scalar.dma_start(fownd[:], fown_b[:])
            nc.scalar.dma_start(fownd1[:], fown_1[:])
            f2own_bcb = cpool.tile([128, NS], BF16)
            nc.scalar.dma_start(f2own_bcb[:], fownd[1:2, :].partition_broadcast(128))
            f1own_bcb = cpool.tile([128, NS], BF16)
            nc.scalar.dma_start(f1own_bcb[:], fownd[0:1, :].partition_broadcast(128))
            f1own_bc64 = cpool.tile([64, NS], F32)
            nc.scalar.dma_start(f1own_bc64[:], fownd1[0:1, :].partition_broadcast(64))

            # ---- x chunks: 1 MB DMAs split across both HWDGE rings ----
            CW = N // NQ  # 4096 columns per chunk
            xs = {}
            for q in range(NQ):
                for c in range(2):
                    xc = cpool.tile([128, CW], BF16, name=f"xs{c}_{q}")
                    eng = nc.sync if c == 0 else nc.scalar
                    eng.dma_start(xc[:], xT[c * 128:(c + 1) * 128,
                                            q * CW:(q + 1) * CW])
                    xs[(c, q)] = xc

            # ---- fused phase A + pass 1 ----
            # per 4-block group: sf GEMM -> bias/copies -> exps -> 4 pass-1 tiles
            sfall = cpool.tile([128, NT, O], BF16)   # sf + b1
            fcols = cpool.tile([128, NT, 2], F32)    # raw f1/f2 columns
            bias4 = bias_bc[:, 0:64].unsqueeze(1).broadcast_to([128, 4, 64])
            a12 = cpool.tile([128, 2, NT], BF16)     # exp(f1+c1), exp(.01(f1+c1))
            a12h = cpool.tile([128, 2, NT], BF16)
            nf1cc = cpool.tile([128, NT], F32)       # -(f1 + c1 + c2)
            pf1cc = cpool.tile([128, NT], F32)       # +(f1 + c1 + c2)
            psum_m = psM.tile([2, NS], F32, tag="m", bufs=1)

            for g in range(16):
                g4 = slice(4 * g, 4 * g + 4)
                ps = psA.tile([128, 4, 66], F32, tag="psA", bufs=2)
                for b4 in range(4):
                    jb = 4 * g + b4
                    q, col = divmod(jb * 128, CW)
                    for c in range(2):
                        nc.tensor.matmul(
                            ps[:, b4, :], xs[(c, q)][:, col:col + 128],
                            movs[:, c, :], start=(c == 0), stop=(c == 1),
                        )
                nc.vector.tensor_tensor(sfall[:, g4, :], ps[:, :, 0:64], bias4, OP.add)
                nc.scalar.activation(fcols[:, g4, :], ps[:, :, 64:66], AF.Copy)
                # exps + thresholds for these 4 tiles
                nc.scalar.activation(
                    a12[:, 0, g4], fcols[:, g4, 0], AF.Exp, bias=c1b
                )
                nc.scalar.activation(
                    a12[:, 1, g4], fcols[:, g4, 0], AF.Exp, scale=0.01, bias=css[:, 0:1]
                )
                nc.vector.tensor_scalar(
                    a12h[:, :, g4], a12[:, :, g4], 0.5, None, OP.mult
                )
                nc.vector.tensor_scalar(
                    nf1cc[:, g4], fcols[:, g4, 0], -1.0, ccb[:], OP.mult, OP.subtract
                )
                nc.vector.tensor_scalar(
                    pf1cc[:, g4], fcols[:, g4, 0], ccb[:], None, OP.add
                )
                # pass-1 tiles for this group
                for b4 in range(4):
                    t = 4 * g + b4
                    on_act = (t % 8) == 5
                    msk = mpool.tile([128, NS], BF16, tag="mask1", bufs=8)
                    if on_act:
                        nc.scalar.activation(
                            msk[:], f2own_bcb[:], AF.Sign, bias=pf1cc[:, t:t + 1]
                        )
                        lhs = a12h[:, :, t]
                    else:
                        nc.vector.tensor_scalar(
                            msk[:], f2own_bcb[:], nf1cc[:, t:t + 1], 0.5,
                            OP.is_ge, OP.subtract,
                        )
                        lhs = a12[:, :, t]
                    for h in range(2):
                        nc.tensor.matmul(
                            psum_m[:, h * 512:(h + 1) * 512],
                            lhs, msk[:, h * 512:(h + 1) * 512],
                            start=(t == 0), stop=(t == NT - 1),
                        )

            # ---- the one real collective: m rows (bf16), triggered ASAP ----
            m_sb = rpool.tile([2, NS], BF16)
            nc.scalar.activation(m_sb[:], psum_m[:], AF.Copy)
            nc.sync.dma_start(ag_in[:], m_sb[:])
            nc.gpsimd.collective_compute(
                "AllGather", OP.bypass, ins=[ag_in[:]], outs=[ag_out[:]],
                replica_groups=[list(range(NCORES))],
            )

            # ---- post-loop: thresholds/exps needed later ----
            nf2cc = cpool.tile([128, NT], F32)
            nc.vector.tensor_scalar(
                nf2cc[:], fcols[:, :, 1], -1.0, ccb[:], OP.mult, OP.subtract
            )
            pf2cc = cpool.tile([128, NT], F32)
            nc.vector.tensor_scalar(pf2cc[:], fcols[:, :, 1], ccb[:], None, OP.add)
            b1cols = cpool.tile([128, NT], F32)      # exp(f2+c2)
            nc.scalar.activation(b1cols[:], fcols[:, :, 1], AF.Exp, bias=c2b)
            b2cols = cpool.tile([128, NT], F32)
            nc.scalar.activation(
                b2cols[:], fcols[:, :, 1], AF.Exp, scale=0.01, bias=css[:, 1:2]
            )

            # SA1/SA2 = full sums of exps (free reduce + partition matmul)
            sa_part = cpool.tile([128, 2], F32)
            nc.vector.reduce_sum(sa_part[:, 0:1], a12[:, 0, :], axis=mybir.AxisListType.X)
            nc.vector.reduce_sum(sa_part[:, 1:2], a12[:, 1, :], axis=mybir.AxisListType.X)
            sa_bf = cpool.tile([128, 2], BF16)
            nc.vector.tensor_copy(sa_bf[:], sa_part[:])
            ps_sa = psT.tile([128, 1], F32, tag="tcol", bufs=1)
            nc.tensor.matmul(ps_sa[0:2, :], sa_bf[:], ones_col[:], start=True, stop=True)
            sa_sb = rpool.tile([2, 1], F32)
            nc.vector.tensor_copy(sa_sb[:], ps_sa[0:2, :])
            nc.sync.dma_start(dsa[:], sa_sb[:])
            sah_bc = cpool.tile([128, 2], F32)
            nc.sync.dma_start(
                sah_bc[:], dsa[:].rearrange("a b -> b a").partition_broadcast(128)
            )
            nc.vector.tensor_scalar(sah_bc[:], sah_bc[:], 0.5, None, OP.mult)

            # prebuild first pass-2 masks (fills DVE/ACT during the collective)
            NPRE = 16
            msk2 = {}

            def build_mask2(t):
                m2 = mpool.tile([128, NS], BF16, tag="mask2", bufs=NPRE)
                if (t % 8) == 5:
                    nc.scalar.activation(
                        m2[:], f1own_bcb[:], AF.Sign, bias=pf2cc[:, t:t + 1]
                    )
                else:
                    nc.vector.tensor_scalar(
                        m2[:], f1own_bcb[:], nf2cc[:, t:t + 1], 0.5,
                        OP.is_ge, OP.subtract,
                    )
                msk2[t] = m2

            for t in range(NPRE):
                build_mask2(t)

            # A exps for the final combine (cheap, off critical path)
            a1i = cpool.tile([64, NS], F32)
            nc.scalar.activation(a1i[:], f1own_bc64[:], AF.Exp, bias=bias_bc[0:64, 64:65])
            a2i = cpool.tile([64, NS], F32)
            nc.scalar.activation(
                a2i[:], f1own_bc64[:], AF.Exp, scale=0.01, bias=css[0:64, 0:1]
            )

            # ---- transpose m rows -> columns via PE, finalize scB (all j) ----
            magg = cpool.tile([16, NS], BF16)
            nc.sync.dma_start(magg[:], ag_out[:])
            trall = psT.tile([128, 8, 8, 2], BF16, tag="tr", bufs=1)
            for b in range(8):
                nc.tensor.transpose(
                    trall[:, b, :, :], magg[:, b * 128:(b + 1) * 128], ident16[:]
                )
            m1v = trall[:, :, :, 0].rearrange("p b k -> p k b")
            m2v = trall[:, :, :, 1].rearrange("p b k -> p k b")

            # D = B1(m1 + SA1/2) + B2(SA2/2 - m2) ; scBk = Bk/D
            u1 = rpool.tile([128, NT], F32)
            nc.vector.tensor_scalar(u1[:], m1v, sah_bc[:, 0:1], None, OP.add)
            nc.vector.tensor_tensor(u1[:], u1[:], b1cols[:], OP.mult)
            u2 = rpool.tile([128, NT], F32)
            nc.vector.tensor_scalar(u2[:], m2v, -1.0, sah_bc[:, 1:2], OP.mult, OP.add)
            nc.vector.tensor_tensor(u2[:], u2[:], b2cols[:], OP.mult)
            nc.vector.tensor_tensor(u1[:], u1[:], u2[:], OP.add)
            rd = rpool.tile([128, NT], F32)
            nc.vector.reciprocal(rd[:], u1[:])
            scb1c = cpool.tile([128, NT], F32)
            nc.vector.tensor_tensor(scb1c[:], b1cols[:], rd[:], OP.mult)
            scb2c = cpool.tile([128, NT], F32)
            nc.vector.tensor_tensor(scb2c[:], b2cols[:], rd[:], OP.mult)
            # bf16 scb with ACT-convention tiles (t%8==5) pre-halved
            scb1m = cpool.tile([128, 8, 8], BF16)
            nc.vector.tensor_copy(scb1m[:].rearrange("p a b -> p (a b)"), scb1c[:])
            nc.vector.tensor_scalar(
                scb1m[:, :, 5:6], scb1m[:, :, 5:6], 0.5, None, OP.mult
            )
            scb2m = cpool.tile([128, 8, 8], BF16)
            nc.vector.tensor_copy(scb2m[:].rearrange("p a b -> p (a b)"), scb2c[:])
            nc.vector.tensor_scalar(
                scb2m[:, :, 5:6], scb2m[:, :, 5:6], 0.5, None, OP.mult
            )

            # q columns per quarter (16 tiles): qa[qq][:, t%16] = [q1 | q2] of tile t
            qas = []
            for qq in range(4):
                qa = cpool.tile([128, 16, 2, 64], BF16, name=f"qa{qq}")
                for s, scbm in ((0, scb1m), (1, scb2m)):
                    sc = scbm[:, 2 * qq:2 * qq + 2, :].rearrange("p a b -> p (a b)")
                    nc.vector.tensor_tensor(
                        qa[:, :, s, :],
                        sfall[:, 16 * qq:16 * qq + 16, :],
                        sc.unsqueeze(2).broadcast_to([128, 16, 64]),
                        OP.mult,
                    )
                qas.append(qa)

            # ---- pass 2: ret^T accumulation over all j ----
            psum_ret = psM.tile([128, NS], F32, tag="ret")
            psum_t = psT.tile([128, 1], F32, tag="tcol", bufs=1)
            for t in range(NT):
                on_act = (t % 8) == 5
                q12 = qas[t // 16][:, t % 16, :, :]
                if t + NPRE < NT:
                    build_mask2(t + NPRE)
                msk = msk2.pop(t)
                for h in range(2):
                    nc.tensor.matmul(
                        psum_ret[:, h * 512:(h + 1) * 512],
                        q12, msk[:, h * 512:(h + 1) * 512],
                        start=(t == 0), stop=(t == NT - 1),
                    )
                nc.tensor.matmul(
                    psum_t[:], q12, ones_col[:] if on_act else halfs_col[:],
                    start=(t == 0), stop=(t == NT - 1),
                )

            # ---- combine + elu (bf16, cross-engine parallel tail) ----
            # R2 side on ACT + DMA shift; R1 side on DVE
            rbig2 = cpool.tile([128, NS], F32)
            nc.scalar.activation(rbig2[64:128, :], psum_ret[64:128, :], AF.Copy)
            r2s = cpool.tile([64, NS], F32)
            nc.sync.dma_start(r2s[:], rbig2[64:128, :])
            tbig = cpool.tile([128, 1], F32)
            nc.scalar.activation(tbig[:], psum_t[:], AF.Copy)
            t2s = cpool.tile([64, 1], F32)
            nc.sync.dma_start(t2s[:], tbig[64:128, :])

            # R1 = psum_ret[0:64] + T1/2 column
            rbig1 = cpool.tile([64, NS], F32)
            nc.vector.tensor_scalar(
                rbig1[:], psum_ret[0:64, :], tbig[0:64, 0:1], None, OP.add
            )
            nc.vector.tensor_tensor(a1i[:], a1i[:], rbig1[:], OP.mult)
            # R2 - T2 side
            nc.vector.tensor_scalar(r2s[:], r2s[:], t2s[:], None, OP.subtract)
            nc.vector.tensor_tensor(a2i[:], a2i[:], r2s[:], OP.mult)
            nc.vector.tensor_tensor(a1i[:], a1i[:], a2i[:], OP.subtract)
            # elu(x) = max(x,0) + min(exp(x)-1, 0); reuse r2s for exp, a2i for max
            nc.scalar.activation(r2s[:], a1i[:], AF.Exp)
            nc.vector.tensor_scalar(a2i[:], a1i[:], 0.0, None, OP.max)
            nc.vector.tensor_scalar(r2s[:], r2s[:], -1.0, 0.0, OP.add, OP.min)
            nc.vector.tensor_tensor(a2i[:], a2i[:], r2s[:], OP.add)
            nc.sync.dma_start(out[:], a2i[:])

    nc.compile()
    return nc


@functools.lru_cache(maxsize=1)
def _get_nc():
    return build_kernel()


def make_in_maps(x, W1, b1, a1, ba1, a2, ba2, **kw):
    import ml_dtypes

    x = np.asarray(x, np.float32)
    W1 = np.asarray(W1, np.float32)
    b1 = np.asarray(b1, np.float32)
    a1 = np.asarray(a1, np.float32)
    a2 = np.asarray(a2, np.float32)
    ba1 = np.asarray(ba1, np.float32)
    ba2 = np.asarray(ba2, np.float32)

    bf = ml_dtypes.bfloat16
    w1t = W1.T                                            # [C, O]
    mov66 = np.concatenate(
        [w1t, (w1t @ a1)[:, None], (w1t @ a2)[:, None]], axis=1
    ).astype(bf)                                          # [C, 66]
    bias66 = np.concatenate(
        [b1, [b1 @ a1 + ba1[0]], [b1 @ a2 + ba2[0]]]
    ).astype(np.float32)[None, :]                         # [1, 66]
    xTf = np.ascontiguousarray(x[0].T).astype(bf)         # [C, N]

    in_maps = []
    for k in range(NCORES):
        in_maps.append({
            "xT": xTf,
            "xoT": np.ascontiguousarray(xTf[:, k * NS:(k + 1) * NS]),
            "mov66": mov66,
            "bias66": bias66,
        })
    return in_maps


def kernel(x, W1, b1, a1, ba1, a2, ba2, **kw):
    in_maps = make_in_maps(x, W1, b1, a1, ba1, a2, ba2)
    res = run_bass_kernel_spmd(_get_nc(), in_maps, core_ids=list(range(NCORES)))
    outp = np.empty((1, N, O), np.float32)
    for k in range(NCORES):
        outp[0, k * NS:(k + 1) * NS, :] = res.results[k]["out"].T
    return outp


# revision 22
# speedup vs baseline: 1.0881x; 1.0881x over previous
"""Trainium2 Bass kernel for nn_Attn_head_9543417332154 (GNN attention head).

Reference computation (B=1, N=8192, C=256, O=64):
    sf[j, o]  = x[j] @ W1.T + b1                    # per-node linear
    f1[i] = sf[i] @ a1 + ba1 ; f2[j] = sf[j] @ a2 + ba2
    logits[i, j] = leaky_relu(f1[i] + f2[j], 0.01)
    coefs = softmax(logits, axis=0 over i)          # nn.Softmax(dim=1)
    ret[i, o] = sum_j coefs[i, j] * sf[j, o] ; out = elu(ret)

Device algorithm (no N x N matrix in HBM, single tiny collective):
  With s = f1[i] + f2[j] and mask = (s >= 0):
    exp(lrelu(s)) = mask * A1[i]B1[j] + (1 - mask) * A2[i]B2[j]
    A1 = exp(f1), B1 = exp(f2), A2 = exp(.01 f1), B2 = exp(.01 f2)
  Every core redundantly computes sf/f1/f2 for ALL nodes from the full x
  (cheap GEMM; removes two collectives and all strided cross-core gathers).
  f1/f2 kept bias-free on chip; the scalar biases c1 = b1@a1+ba1 and
  c2 = b1@a2+ba2 are folded into the exp biases and mask thresholds.
  Pass 1 (own j-shard, all i) is INTERLEAVED with the sf GEMM per 4-block
  group to keep the PE dense, psum convention +-1/2:
    m[j] = sum_i (mask - 1/2) A[i]  =  M[j] - SA/2   (M = masked sum, SA = full)
  ONE AllGather of m rows (bf16) [2, 1024] -> [16, 1024]; PE-transpose to cols;
    D[j] = B1 M1 + B2 (SA2 - M2);  scBk[j] = Bk[j] / D[j]   (all j, every core)
  Pass 2 (own i-shard, all j), same +-1/2 mask trick:
    Qk[j, o] = scBk[j] * sf[j, o]
    psum_ret = sum_j (mask - 1/2) Q ; psum_t = sum_j Q/2
    R1 = psum_ret[0:64] + T1/2 ; R2 - T2 = psum_ret[64:128] - T2/2
    ret[i, o] = A1[i] R1 - A2[i] (R2 - T2);  out = elu(ret)

Sharding: output rows (own i) sharded 1024/core; everything else replicated.
"""
import functools
import numpy as np

import concourse.bass as bass
import concourse.bacc as bacc
import concourse.tile as tile
import concourse.mybir as mybir
from concourse.bass_utils import run_bass_kernel_spmd
from concourse.masks import make_identity

F32 = mybir.dt.float32
BF16 = mybir.dt.bfloat16
AF = mybir.ActivationFunctionType
OP = mybir.AluOpType

NCORES = 8
N = 8192
C = 256
O = 64
NS = N // NCORES           # 1024 rows per core
T = NS // 128              # 8 local 128-row tiles
NT = N // 128              # 64 global 128-row tiles
NQ = 2                     # x load column chunks per c-half (1 MB each)


def build_kernel():
    nc = bacc.Bacc(None, target_bir_lowering=False)

    xT = nc.dram_tensor("xT", [C, N], BF16, kind="ExternalInput")       # full x^T
    xoT = nc.dram_tensor("xoT", [C, NS], BF16, kind="ExternalInput")    # own shard x^T
    mov66 = nc.dram_tensor("mov66", [C, 66], BF16, kind="ExternalInput")
    bias66 = nc.dram_tensor("bias66", [1, 66], F32, kind="ExternalInput")
    out = nc.dram_tensor("out", [O, NS], F32, kind="ExternalOutput")    # ret^T shard

    with tile.TileContext(nc) as tc:
        with (
            tc.tile_pool(name="const", bufs=1) as cpool,
            tc.tile_pool(name="work", bufs=3) as wpool,
            tc.tile_pool(name="rows", bufs=1) as rpool,
            tc.tile_pool(name="mask", bufs=4) as mpool,
            tc.tile_pool(name="psA", bufs=2, space="PSUM") as psA,
            tc.tile_pool(name="psM", bufs=1, space="PSUM") as psM,
            tc.tile_pool(name="psT", bufs=1, space="PSUM") as psT,
            tc.tile_pool(name="dram", bufs=1, space="DRAM") as dram,
        ):
            # ---- DRAM buffers ----
            ag0_in = dram.tile([1, 1], F32)
            ag0_out = dram.tile([NCORES, 1], F32, addr_space="Shared")
            fownd = dram.tile([2, NS], BF16)      # raw f1own/f2own rows
            fownd1 = dram.tile([1, NS], F32)      # raw f1own row f32
            dsa = dram.tile([2, 1], F32)          # SA1, SA2
            ag_in = dram.tile([2, NS], BF16)      # m rows (M - SA/2)
            ag_out = dram.tile([2 * NCORES, NS], BF16, addr_space="Shared")

            # ---- constants ----
            ones_col = cpool.tile([128, 1], BF16)
            nc.gpsimd.memset(ones_col[:], 1.0)
            halfs_col = cpool.tile([128, 1], BF16)
            nc.gpsimd.memset(halfs_col[:], 0.5)
            ident16 = cpool.tile([16, 16], BF16)
            make_identity(nc, ident16[:])

            # dummy collective: absorbs CC-stream warmup + first-op latency
            zz = cpool.tile([1, 1], F32)
            nc.gpsimd.memset(zz[:], 0.0)
            nc.sync.dma_start(ag0_in[:], zz[:])
            nc.gpsimd.collective_compute(
                "AllGather", OP.bypass, ins=[ag0_in[:]], outs=[ag0_out[:]],
                replica_groups=[list(range(NCORES))],
            )

            movs = cpool.tile([128, 2, 66], BF16)
            for c in range(2):
                nc.scalar.dma_start(movs[:, c, :], mov66[c * 128:(c + 1) * 128, :])
            bias_bc = cpool.tile([128, 66], F32)
            nc.scalar.dma_start(bias_bc[:], bias66[0:1, :].partition_broadcast(128))
            c1b = bias_bc[:, 64:65]
            c2b = bias_bc[:, 65:66]
            css = cpool.tile([128, 2], F32)       # 0.01*c1, 0.01*c2
            nc.vector.tensor_scalar(css[:], bias_bc[:, 64:66], 0.01, None, OP.mult)
            ccb = cpool.tile([128, 1], F32)       # c1 + c2
            nc.vector.tensor_tensor(ccb[:], c1b, c2b, OP.add)

            # ---- phase A0: own-shard raw f1/f2 rows (for mask broadcasts) ----
            xo = cpool.tile([128, 2, NS], BF16)
            for c in range(2):
                nc.scalar.dma_start(xo[:, c, :], xoT[c * 128:(c + 1) * 128, :])
            psF = psM.tile([2, NS], F32, tag="m", bufs=1)  # same slot as psum_m
            for h in range(2):
                sl = slice(h * 512, (h + 1) * 512)
                for c in range(2):
                    nc.tensor.matmul(
                        psF[:, sl], movs[:, c, 64:66], xo[:, c, sl],
                        start=(c == 0), stop=(c == 1),
                    )
            fown_b = rpool.tile([2, NS], BF16)
            nc.vector.tensor_copy(fown_b[:], psF[:])
            fown_1 = rpool.tile([1, NS], F32)
            nc.scalar.activation(fown_1[:], psF[0:1, :], AF.Copy)
            # ---- x chunks split across both HWDGE rings; fown-chain DMAs are
            # interleaved on the sync ring so neither ring stalls the other
            CW = N // NQ
            xs = {}

            def xchunk(c, q):
                xc = cpool.tile([128, CW], BF16, name=f"xs{c}_{q}")
                eng = nc.sync if c == 0 else nc.scalar
                eng.dma_start(xc[:], xT[c * 128:(c + 1) * 128,
                                        q * CW:(q + 1) * CW])
                xs[(c, q)] = xc

            xchunk(0, 0)
            xchunk(1, 0)
            nc.sync.dma_start(fownd[:], fown_b[:])
            f2own_bcb = cpool.tile([128, NS], BF16)
            nc.sync.dma_start(f2own_bcb[:], fownd[1:2, :].partition_broadcast(128))
            xchunk(0, 1)
            xchunk(1, 1)
            nc.sync.dma_start(fownd1[:], fown_1[:])
            f1own_bcb = cpool.tile([128, NS], BF16)
            nc.sync.dma_start(f1own_bcb[:], fownd[0:1, :].partition_broadcast(128))
            f1own_bc128 = cpool.tile([128, NS], F32)
            nc.sync.dma_start(f1own_bc128[:], fownd1[0:1, :].partition_broadcast(128))

            # ---- fused phase A + pass 1 ----
            # per 4-block group: sf GEMM -> bias/copies -> exps -> 4 pass-1 tiles
            sfall = cpool.tile([128, NT, O], BF16)   # sf + b1
            fcols = cpool.tile([128, NT, 2], F32)    # raw f1/f2 columns
            bias4 = bias_bc[:, 0:64].unsqueeze(1).broadcast_to([128, 4, 64])
            a12 = cpool.tile([128, 2, NT], BF16)     # exp(f1+c1), exp(.01(f1+c1))
            a12h = cpool.tile([128, 2, NT], BF16)
            nf1cc = cpool.tile([128, NT], F32)       # -(f1 + c1 + c2)
            pf1cc = cpool.tile([128, NT], F32)       # +(f1 + c1 + c2)
            psum_m = psM.tile([2, NS], F32, tag="m", bufs=1)

            for g in range(16):
                g4 = slice(4 * g, 4 * g + 4)
                ps = psA.tile([128, 4, 66], F32, tag="psA", bufs=2)
                for b4 in range(4):
                    jb = 4 * g + b4
                    q, col = divmod(jb * 128, CW)
                    for c in range(2):
                        nc.tensor.matmul(
                            ps[:, b4, :], xs[(c, q)][:, col:col + 128],
                            movs[:, c, :], start=(c == 0), stop=(c == 1),
                        )
                nc.vector.tensor_tensor(sfall[:, g4, :], ps[:, :, 0:64], bias4, OP.add)
                nc.scalar.activation(fcols[:, g4, :], ps[:, :, 64:66], AF.Copy)
                # exps + thresholds for these 4 tiles
                nc.scalar.activation(
                    a12[:, 0, g4], fcols[:, g4, 0], AF.Exp, bias=c1b
                )
                nc.scalar.activation(
                    a12[:, 1, g4], fcols[:, g4, 0], AF.Exp, scale=0.01, bias=css[:, 0:1]
                )
                nc.vector.tensor_scalar(
                    a12h[:, :, g4], a12[:, :, g4], 0.5, None, OP.mult
                )
                nc.vector.tensor_scalar(
                    nf1cc[:, g4], fcols[:, g4, 0], -1.0, ccb[:], OP.mult, OP.subtract
                )
                nc.vector.tensor_scalar(
                    pf1cc[:, g4], fcols[:, g4, 0], ccb[:], None, OP.add
                )
                # pass-1 tiles for this group
                for b4 in range(4):
                    t = 4 * g + b4
                    on_act = (t % 8) == 5
                    msk = mpool.tile([128, NS], BF16, tag="mask1", bufs=8)
                    if on_act:
                        nc.scalar.activation(
                            msk[:], f2own_bcb[:], AF.Sign, bias=pf1cc[:, t:t + 1]
                        )
                        lhs = a12h[:, :, t]
                    else:
                        nc.vector.tensor_scalar(
                            msk[:], f2own_bcb[:], nf1cc[:, t:t + 1], 0.5,
                            OP.is_ge, OP.subtract,
                        )
                        lhs = a12[:, :, t]
                    for h in range(2):
                        nc.tensor.matmul(
                            psum_m[:, h * 512:(h + 1) * 512],
                            lhs, msk[:, h * 512:(h + 1) * 512],
                            start=(t == 0), stop=(t == NT - 1),
                        )

            # ---- the one real collective: m rows (bf16), triggered ASAP ----
            m_sb = rpool.tile([2, NS], BF16)
            nc.scalar.activation(m_sb[:], psum_m[:], AF.Copy)
            nc.sync.dma_start(ag_in[:], m_sb[:])
            nc.gpsimd.collective_compute(
                "AllGather", OP.bypass, ins=[ag_in[:]], outs=[ag_out[:]],
                replica_groups=[list(range(NCORES))],
            )

            # ---- post-loop: thresholds/exps needed later ----
            nf2cc = cpool.tile([128, NT], F32)
            nc.vector.tensor_scalar(
                nf2cc[:], fcols[:, :, 1], -1.0, ccb[:], OP.mult, OP.subtract
            )
            pf2cc = cpool.tile([128, NT], F32)
            nc.vector.tensor_scalar(pf2cc[:], fcols[:, :, 1], ccb[:], None, OP.add)
            b1cols = cpool.tile([128, NT], F32)      # exp(f2+c2)
            nc.scalar.activation(b1cols[:], fcols[:, :, 1], AF.Exp, bias=c2b)
            b2cols = cpool.tile([128, NT], F32)
            nc.scalar.activation(
                b2cols[:], fcols[:, :, 1], AF.Exp, scale=0.01, bias=css[:, 1:2]
            )

            # SA1/SA2 = full sums of exps (free reduce + partition matmul)
            sa_part = cpool.tile([128, 2], F32)
            nc.vector.reduce_sum(sa_part[:, 0:1], a12[:, 0, :], axis=mybir.AxisListType.X)
            nc.vector.reduce_sum(sa_part[:, 1:2], a12[:, 1, :], axis=mybir.AxisListType.X)
            sa_bf = cpool.tile([128, 2], BF16)
            nc.vector.tensor_copy(sa_bf[:], sa_part[:])
            ps_sa = psT.tile([128, 1], F32, tag="tcol", bufs=1)
            nc.tensor.matmul(ps_sa[0:2, :], sa_bf[:], ones_col[:], start=True, stop=True)
            sa_sb = rpool.tile([2, 1], F32)
            nc.vector.tensor_copy(sa_sb[:], ps_sa[0:2, :])
            nc.sync.dma_start(dsa[:], sa_sb[:])
            sah_bc = cpool.tile([128, 2], F32)
            nc.sync.dma_start(
                sah_bc[:], dsa[:].rearrange("a b -> b a").partition_broadcast(128)
            )
            nc.vector.tensor_scalar(sah_bc[:], sah_bc[:], 0.5, None, OP.mult)

            # prebuild first pass-2 masks (fills DVE/ACT during the collective)
            NPRE = 16
            msk2 = {}

            def build_mask2(t):
                m2 = mpool.tile([128, NS], BF16, tag="mask2", bufs=NPRE)
                if (t % 8) == 5:
                    nc.scalar.activation(
                        m2[:], f1own_bcb[:], AF.Sign, bias=pf2cc[:, t:t + 1]
                    )
                else:
                    nc.vector.tensor_scalar(
                        m2[:], f1own_bcb[:], nf2cc[:, t:t + 1], 0.5,
                        OP.is_ge, OP.subtract,
                    )
                msk2[t] = m2

            for t in range(NPRE):
                build_mask2(t)

            # A exps packed [A1; A2] for the final combine (off critical path)
            aio = cpool.tile([128, NS], F32)
            nc.scalar.activation(
                aio[0:64, :], f1own_bc128[0:64, :], AF.Exp, bias=bias_bc[0:64, 64:65]
            )
            nc.scalar.activation(
                aio[64:128, :], f1own_bc128[64:128, :], AF.Exp, scale=0.01,
                bias=css[64:128, 0:1]
            )

            # ---- transpose m rows -> columns via PE, finalize scB (all j) ----
            magg = cpool.tile([16, NS], BF16)
            nc.sync.dma_start(magg[:], ag_out[:])
            trall = psT.tile([128, 8, 8, 2], BF16, tag="tr", bufs=1)
            for b in range(8):
                nc.tensor.transpose(
                    trall[:, b, :, :], magg[:, b * 128:(b + 1) * 128], ident16[:]
                )
            m1v = trall[:, :, :, 0].rearrange("p b k -> p k b")
            m2v = trall[:, :, :, 1].rearrange("p b k -> p k b")

            # D = B1(m1 + SA1/2) + B2(SA2/2 - m2) ; scBk = Bk/D
            u1 = rpool.tile([128, NT], F32)
            nc.vector.tensor_scalar(u1[:], m1v, sah_bc[:, 0:1], None, OP.add)
            nc.vector.tensor_tensor(u1[:], u1[:], b1cols[:], OP.mult)
            u2 = rpool.tile([128, NT], F32)
            nc.vector.tensor_scalar(u2[:], m2v, -1.0, sah_bc[:, 1:2], OP.mult, OP.add)
            nc.vector.tensor_tensor(u2[:], u2[:], b2cols[:], OP.mult)
            nc.vector.tensor_tensor(u1[:], u1[:], u2[:], OP.add)
            rd = rpool.tile([128, NT], F32)
            nc.vector.reciprocal(rd[:], u1[:])
            scb1c = cpool.tile([128, NT], F32)
            nc.vector.tensor_tensor(scb1c[:], b1cols[:], rd[:], OP.mult)
            scb2c = cpool.tile([128, NT], F32)
            nc.vector.tensor_tensor(scb2c[:], b2cols[:], rd[:], OP.mult)
            # bf16 scb with ACT-convention tiles (t%8==5) pre-halved
            scb1m = cpool.tile([128, 8, 8], BF16)
            nc.vector.tensor_copy(scb1m[:].rearrange("p a b -> p (a b)"), scb1c[:])
            nc.vector.tensor_scalar(
                scb1m[:, :, 5:6], scb1m[:, :, 5:6], 0.5, None, OP.mult
            )
            scb2m = cpool.tile([128, 8, 8], BF16)
            nc.vector.tensor_copy(scb2m[:].rearrange("p a b -> p (a b)"), scb2c[:])
            nc.vector.tensor_scalar(
                scb2m[:, :, 5:6], scb2m[:, :, 5:6], 0.5, None, OP.mult
            )

            # q columns per quarter (16 tiles): qa[qq][:, t%16] = [q1 | q2] of tile t
            qas = []
            for qq in range(4):
                qa = cpool.tile([128, 16, 2, 64], BF16, name=f"qa{qq}")
                for s, scbm in ((0, scb1m), (1, scb2m)):
                    sc = scbm[:, 2 * qq:2 * qq + 2, :].rearrange("p a b -> p (a b)")
                    nc.vector.tensor_tensor(
                        qa[:, :, s, :],
                        sfall[:, 16 * qq:16 * qq + 16, :],
                        sc.unsqueeze(2).broadcast_to([128, 16, 64]),
                        OP.mult,
                    )
                qas.append(qa)

            # ---- pass 2: ret^T accumulation over all j ----
            psum_ret = psM.tile([128, NS], F32, tag="ret")
            psum_t = psT.tile([128, 1], F32, tag="tcol", bufs=1)
            for t in range(NT):
                on_act = (t % 8) == 5
                q12 = qas[t // 16][:, t % 16, :, :]
                if t + NPRE < NT:
                    build_mask2(t + NPRE)
                msk = msk2.pop(t)
                for h in range(2):
                    nc.tensor.matmul(
                        psum_ret[:, h * 512:(h + 1) * 512],
                        q12, msk[:, h * 512:(h + 1) * 512],
                        start=(t == 0), stop=(t == NT - 1),
                    )
                nc.tensor.matmul(
                    psum_t[:], q12, ones_col[:] if on_act else halfs_col[:],
                    start=(t == 0), stop=(t == NT - 1),
                )

            # ---- combine + elu (packed 128-partition tail) ----
            # tfix = [+T1/2 (rows 0:64); -T2/2 (rows 64:128)]
            tbig = cpool.tile([128, 1], F32)
            nc.vector.tensor_copy(tbig[0:64, :], psum_t[0:64, :])
            nc.vector.tensor_scalar(
                tbig[64:128, :], psum_t[64:128, :], -1.0, None, OP.mult
            )
            # rbigF = [R1 ; R2 - T2] = psum_ret + tfix, then prod = [A1 R1; A2(R2-T2)]
            rbigF = cpool.tile([128, NS], F32)
            nc.vector.tensor_scalar(
                rbigF[:], psum_ret[:], tbig[:, 0:1], None, OP.add
            )
            nc.vector.tensor_tensor(rbigF[:], rbigF[:], aio[:], OP.mult)
            p2s = cpool.tile([64, NS], F32)
            nc.sync.dma_start(p2s[:], rbigF[64:128, :])
            ret = cpool.tile([64, NS], F32)
            nc.vector.tensor_tensor(ret[:], rbigF[0:64, :], p2s[:], OP.subtract)
            # elu(x) = max(x,0) + min(exp(x)-1, 0)
            ex = cpool.tile([64, NS], F32)
            nc.scalar.activation(ex[:], ret[:], AF.Exp)
            nc.vector.tensor_scalar(ret[:], ret[:], 0.0, None, OP.max)
            nc.vector.tensor_scalar(ex[:], ex[:], -1.0, 0.0, OP.add, OP.min)
            nc.vector.tensor_tensor(ret[:], ret[:], ex[:], OP.add)
            nc.sync.dma_start(out[:], ret[:])

    nc.compile()
    return nc


@functools.lru_cache(maxsize=1)
def _get_nc():
    return build_kernel()


def make_in_maps(x, W1, b1, a1, ba1, a2, ba2, **kw):
    import ml_dtypes

    x = np.asarray(x, np.float32)
    W1 = np.asarray(W1, np.float32)
    b1 = np.asarray(b1, np.float32)
    a1 = np.asarray(a1, np.float32)
    a2 = np.asarray(a2, np.float32)
    ba1 = np.asarray(ba1, np.float32)
    ba2 = np.asarray(ba2, np.float32)

    bf = ml_dtypes.bfloat16
    w1t = W1.T                                            # [C, O]
    mov66 = np.concatenate(
        [w1t, (w1t @ a1)[:, None], (w1t @ a2)[:, None]], axis=1
    ).astype(bf)                                          # [C, 66]
    bias66 = np.concatenate(
        [b1, [b1 @ a1 + ba1[0]], [b1 @ a2 + ba2[0]]]
    ).astype(np.float32)[None, :]                         # [1, 66]
    xTf = np.ascontiguousarray(x[0].T).astype(bf)         # [C, N]

    in_maps = []
    for k in range(NCORES):
        in_maps.append({
            "xT": xTf,
            "xoT": np.ascontiguousarray(xTf[:, k * NS:(k + 1) * NS]),
            "mov66": mov66,
            "bias66": bias66,
        })
    return in_maps


def kernel(x, W1, b1, a1, ba1, a2, ba2, **kw):
    in_maps = make_in_maps(x, W1, b1, a1, ba1, a2, ba2)
    res = run_bass_kernel_spmd(_get_nc(), in_maps, core_ids=list(range(NCORES)))
    outp = np.empty((1, N, O), np.float32)
    for k in range(NCORES):
        outp[0, k * NS:(k + 1) * NS, :] = res.results[k]["out"].T
    return outp


# revision 25
# speedup vs baseline: 1.1782x; 1.0828x over previous
"""Trainium2 Bass kernel for nn_Attn_head_9543417332154 (GNN attention head).

Reference computation (B=1, N=8192, C=256, O=64):
    sf[j, o]  = x[j] @ W1.T + b1                    # per-node linear
    f1[i] = sf[i] @ a1 + ba1 ; f2[j] = sf[j] @ a2 + ba2
    logits[i, j] = leaky_relu(f1[i] + f2[j], 0.01)
    coefs = softmax(logits, axis=0 over i)          # nn.Softmax(dim=1)
    ret[i, o] = sum_j coefs[i, j] * sf[j, o] ; out = elu(ret)

Device algorithm (no N x N matrix in HBM, single tiny collective):
  With s = f1[i] + f2[j] and mask = (s >= 0):
    exp(lrelu(s)) = mask * A1[i]B1[j] + (1 - mask) * A2[i]B2[j]
    A1 = exp(f1), B1 = exp(f2), A2 = exp(.01 f1), B2 = exp(.01 f2)
  Every core redundantly computes sf/f1/f2 for ALL nodes from the full x
  (cheap GEMM; removes two collectives and all strided cross-core gathers).
  f1/f2 kept bias-free on chip; the scalar biases c1 = b1@a1+ba1 and
  c2 = b1@a2+ba2 are folded into the exp biases and mask thresholds.
  Pass 1 (own j-shard, all i) is INTERLEAVED with the sf GEMM per 4-block
  group to keep the PE dense, psum convention +-1/2:
    m[j] = sum_i (mask - 1/2) A[i]  =  M[j] - SA/2   (M = masked sum, SA = full)
  ONE AllGather of m rows (bf16) [2, 1024] -> [16, 1024]; PE-transpose to cols;
    D[j] = B1 M1 + B2 (SA2 - M2);  scBk[j] = Bk[j] / D[j]   (all j, every core)
  Pass 2 (own i-shard, all j), same +-1/2 mask trick:
    Qk[j, o] = scBk[j] * sf[j, o]
    psum_ret = sum_j (mask - 1/2) Q ; psum_t = sum_j Q/2
    R1 = psum_ret[0:64] + T1/2 ; R2 - T2 = psum_ret[64:128] - T2/2
    ret[i, o] = A1[i] R1 - A2[i] (R2 - T2);  out = elu(ret)

Sharding: output rows (own i) sharded 1024/core; everything else replicated.
"""
import functools
import numpy as np

import concourse.bass as bass
import concourse.bacc as bacc
import concourse.tile as tile
import concourse.mybir as mybir
from concourse.bass_utils import run_bass_kernel_spmd
from concourse.masks import make_identity

F32 = mybir.dt.float32
BF16 = mybir.dt.bfloat16
AF = mybir.ActivationFunctionType
OP = mybir.AluOpType

NCORES = 8
N = 8192
C = 256
O = 64
NS = N // NCORES           # 1024 rows per core
T = NS // 128              # 8 local 128-row tiles
NT = N // 128              # 64 global 128-row tiles
NQ = 2                     # x load column chunks per c-half (1 MB each)


def build_kernel():
    nc = bacc.Bacc(None, target_bir_lowering=False)

    xT = nc.dram_tensor("xT", [C, N], BF16, kind="ExternalInput")       # full x^T
    xoT = nc.dram_tensor("xoT", [C, NS], BF16, kind="ExternalInput")    # own shard x^T
    mov66 = nc.dram_tensor("mov66", [C, 66], BF16, kind="ExternalInput")
    bias66 = nc.dram_tensor("bias66", [1, 66], F32, kind="ExternalInput")
    out = nc.dram_tensor("out", [O, NS], F32, kind="ExternalOutput")    # ret^T shard

    with tile.TileContext(nc) as tc:
        with (
            tc.tile_pool(name="const", bufs=1) as cpool,
            tc.tile_pool(name="work", bufs=3) as wpool,
            tc.tile_pool(name="rows", bufs=1) as rpool,
            tc.tile_pool(name="mask", bufs=4) as mpool,
            tc.tile_pool(name="psA", bufs=2, space="PSUM") as psA,
            tc.tile_pool(name="psM", bufs=1, space="PSUM") as psM,
            tc.tile_pool(name="psT", bufs=1, space="PSUM") as psT,
            tc.tile_pool(name="dram", bufs=1, space="DRAM") as dram,
        ):
            # ---- DRAM buffers ----
            ag0_in = dram.tile([1, 1], F32)
            ag0_out = dram.tile([NCORES, 1], F32, addr_space="Shared")
            dsa = dram.tile([2, 1], F32)          # SA1, SA2
            ag_in = dram.tile([2, NS], BF16)      # m rows (M - SA/2)
            ag_out = dram.tile([2 * NCORES, NS], BF16, addr_space="Shared")

            # ---- constants ----
            ones_col = cpool.tile([128, 1], BF16)
            nc.gpsimd.memset(ones_col[:], 1.0)
            halfs_col = cpool.tile([128, 1], BF16)
            nc.gpsimd.memset(halfs_col[:], 0.5)
            ident16 = cpool.tile([16, 16], BF16)
            make_identity(nc, ident16[:])

            # dummy collective: absorbs CC-stream warmup + first-op latency
            zz = cpool.tile([1, 1], F32)
            nc.gpsimd.memset(zz[:], 0.0)
            nc.sync.dma_start(ag0_in[:], zz[:])
            nc.gpsimd.collective_compute(
                "AllGather", OP.bypass, ins=[ag0_in[:]], outs=[ag0_out[:]],
                replica_groups=[list(range(NCORES))],
            )

            movs = cpool.tile([128, 2, 66], BF16)
            for c in range(2):
                nc.scalar.dma_start(movs[:, c, :], mov66[c * 128:(c + 1) * 128, :])
            bias_bc = cpool.tile([128, 66], F32)
            nc.scalar.dma_start(bias_bc[:], bias66[0:1, :].partition_broadcast(128))
            c1b = bias_bc[:, 64:65]
            c2b = bias_bc[:, 65:66]
            css = cpool.tile([128, 2], F32)       # 0.01*c1, 0.01*c2
            nc.vector.tensor_scalar(css[:], bias_bc[:, 64:66], 0.01, None, OP.mult)
            ccb = cpool.tile([128, 1], F32)       # c1 + c2
            nc.vector.tensor_tensor(ccb[:], c1b, c2b, OP.add)

            # ---- input DMAs: xo halves + x chunks split across both rings ----
            xo = cpool.tile([128, 2, NS], BF16)
            nc.sync.dma_start(xo[:, 0, :], xoT[0:128, :])
            nc.scalar.dma_start(xo[:, 1, :], xoT[128:256, :])
            CW = N // NQ
            xs = {}
            for q in range(NQ):
                for c in range(2):
                    xc = cpool.tile([128, CW], BF16, name=f"xs{c}_{q}")
                    eng = nc.sync if c == 0 else nc.scalar
                    eng.dma_start(xc[:], xT[c * 128:(c + 1) * 128,
                                            q * CW:(q + 1) * CW])
                    xs[(c, q)] = xc

            # ---- phase A0: own-shard raw f1/f2 rows; broadcasts via PE rank-1 ----
            psF = psM.tile([2, NS], F32, tag="m", bufs=1)  # same slot as psum_m
            for h in range(2):
                sl = slice(h * 512, (h + 1) * 512)
                for c in range(2):
                    nc.tensor.matmul(
                        psF[:, sl], movs[:, c, 64:66], xo[:, c, sl],
                        start=(c == 0), stop=(c == 1),
                    )
            fown_b = rpool.tile([2, NS], BF16)
            nc.vector.tensor_copy(fown_b[:], psF[:])
            # selector weights: sel[:, 0, :] picks f1 row, sel[:, 1, :] picks f2
            sel2 = cpool.tile([2, 2, 128], BF16)
            nc.gpsimd.memset(sel2[:], 0.0)
            # sel2[p, k, :] = 1.0 where k == p (affine iota = k - p)
            nc.gpsimd.affine_select(
                out=sel2[:], in_=sel2[:], compare_op=OP.not_equal, fill=1.0,
                base=0, channel_multiplier=-1, pattern=[[1, 2], [0, 128]],
            )
            psB = psM.tile([128, NS], F32, tag="ret", bufs=1)
            for h in range(2):
                sl = slice(h * 512, (h + 1) * 512)
                nc.tensor.matmul(psB[:, sl], sel2[:, 1, :], fown_b[:, sl],
                                 start=True, stop=True)
            f2own_bcb = cpool.tile([128, NS], BF16)
            nc.vector.tensor_copy(f2own_bcb[:], psB[:])
            psB2 = psM.tile([128, NS], F32, tag="ret", bufs=1)
            for h in range(2):
                sl = slice(h * 512, (h + 1) * 512)
                nc.tensor.matmul(psB2[:, sl], sel2[:, 0, :], fown_b[:, sl],
                                 start=True, stop=True)
            f1own_bcb = cpool.tile([128, NS], BF16)
            nc.vector.tensor_copy(f1own_bcb[:], psB2[:])
            f1own_bc128 = cpool.tile([128, NS], F32)
            nc.scalar.activation(f1own_bc128[:], psB2[:], AF.Copy)

            # ---- fused phase A + pass 1 ----
            # per 4-block group: sf GEMM -> bias/copies -> exps -> 4 pass-1 tiles
            sfall = cpool.tile([128, NT, O], BF16)   # sf + b1
            fcols = cpool.tile([128, NT, 2], F32)    # raw f1/f2 columns
            bias4 = bias_bc[:, 0:64].unsqueeze(1).broadcast_to([128, 4, 64])
            a12 = cpool.tile([128, 2, NT], BF16)     # exp(f1+c1), exp(.01(f1+c1))
            a12h = cpool.tile([128, 2, NT], BF16)
            nf1cc = cpool.tile([128, NT], F32)       # -(f1 + c1 + c2)
            pf1cc = cpool.tile([128, NT], F32)       # +(f1 + c1 + c2)
            psum_m = psM.tile([2, NS], F32, tag="m", bufs=1)

            for g in range(16):
                g4 = slice(4 * g, 4 * g + 4)
                ps = psA.tile([128, 4, 66], F32, tag="psA", bufs=2)
                for b4 in range(4):
                    jb = 4 * g + b4
                    q, col = divmod(jb * 128, CW)
                    for c in range(2):
                        nc.tensor.matmul(
                            ps[:, b4, :], xs[(c, q)][:, col:col + 128],
                            movs[:, c, :], start=(c == 0), stop=(c == 1),
                        )
                nc.vector.tensor_tensor(sfall[:, g4, :], ps[:, :, 0:64], bias4, OP.add)
                nc.scalar.activation(fcols[:, g4, :], ps[:, :, 64:66], AF.Copy)
                # exps + thresholds for these 4 tiles
                nc.scalar.activation(
                    a12[:, 0, g4], fcols[:, g4, 0], AF.Exp, bias=c1b
                )
                nc.scalar.activation(
                    a12[:, 1, g4], fcols[:, g4, 0], AF.Exp, scale=0.01, bias=css[:, 0:1]
                )
                nc.vector.tensor_scalar(
                    a12h[:, :, g4], a12[:, :, g4], 0.5, None, OP.mult
                )
                nc.vector.tensor_scalar(
                    nf1cc[:, g4], fcols[:, g4, 0], -1.0, ccb[:], OP.mult, OP.subtract
                )
                nc.vector.tensor_scalar(
                    pf1cc[:, g4], fcols[:, g4, 0], ccb[:], None, OP.add
                )
                # pass-1 tiles for this group
                for b4 in range(4):
                    t = 4 * g + b4
                    on_act = (t % 8) == 5
                    msk = mpool.tile([128, NS], BF16, tag="mask1", bufs=8)
                    if on_act:
                        nc.scalar.activation(
                            msk[:], f2own_bcb[:], AF.Sign, bias=pf1cc[:, t:t + 1]
                        )
                        lhs = a12h[:, :, t]
                    else:
                        nc.vector.tensor_scalar(
                            msk[:], f2own_bcb[:], nf1cc[:, t:t + 1], 0.5,
                            OP.is_ge, OP.subtract,
                        )
                        lhs = a12[:, :, t]
                    for h in range(2):
                        nc.tensor.matmul(
                            psum_m[:, h * 512:(h + 1) * 512],
                            lhs, msk[:, h * 512:(h + 1) * 512],
                            start=(t == 0), stop=(t == NT - 1),
                        )

            # ---- the one real collective: m rows (bf16), triggered ASAP ----
            m_sb = rpool.tile([2, NS], BF16)
            nc.scalar.activation(m_sb[:], psum_m[:], AF.Copy)
            nc.sync.dma_start(ag_in[:], m_sb[:])
            nc.gpsimd.collective_compute(
                "AllGather", OP.bypass, ins=[ag_in[:]], outs=[ag_out[:]],
                replica_groups=[list(range(NCORES))],
            )

            # ---- post-loop: thresholds/exps needed later ----
            nf2cc = cpool.tile([128, NT], F32)
            nc.vector.tensor_scalar(
                nf2cc[:], fcols[:, :, 1], -1.0, ccb[:], OP.mult, OP.subtract
            )
            pf2cc = cpool.tile([128, NT], F32)
            nc.vector.tensor_scalar(pf2cc[:], fcols[:, :, 1], ccb[:], None, OP.add)
            b1cols = cpool.tile([128, NT], F32)      # exp(f2+c2)
            nc.scalar.activation(b1cols[:], fcols[:, :, 1], AF.Exp, bias=c2b)
            b2cols = cpool.tile([128, NT], F32)
            nc.scalar.activation(
                b2cols[:], fcols[:, :, 1], AF.Exp, scale=0.01, bias=css[:, 1:2]
            )

            # SA1/SA2 = full sums of exps (free reduce + partition matmul)
            sa_part = cpool.tile([128, 2], F32)
            nc.vector.reduce_sum(sa_part[:, 0:1], a12[:, 0, :], axis=mybir.AxisListType.X)
            nc.vector.reduce_sum(sa_part[:, 1:2], a12[:, 1, :], axis=mybir.AxisListType.X)
            sa_bf = cpool.tile([128, 2], BF16)
            nc.vector.tensor_copy(sa_bf[:], sa_part[:])
            ps_sa = psT.tile([128, 1], F32, tag="tcol", bufs=1)
            nc.tensor.matmul(ps_sa[0:2, :], sa_bf[:], ones_col[:], start=True, stop=True)
            sa_sb = rpool.tile([2, 1], F32)
            nc.vector.tensor_copy(sa_sb[:], ps_sa[0:2, :])
            nc.sync.dma_start(dsa[:], sa_sb[:])
            sah_bc = cpool.tile([128, 2], F32)
            nc.sync.dma_start(
                sah_bc[:], dsa[:].rearrange("a b -> b a").partition_broadcast(128)
            )
            nc.vector.tensor_scalar(sah_bc[:], sah_bc[:], 0.5, None, OP.mult)

            # prebuild first pass-2 masks (fills DVE/ACT during the collective)
            NPRE = 16
            msk2 = {}

            def build_mask2(t):
                m2 = mpool.tile([128, NS], BF16, tag="mask2", bufs=NPRE)
                nc.vector.tensor_scalar(
                    m2[:], f1own_bcb[:], nf2cc[:, t:t + 1], 0.5,
                    OP.is_ge, OP.subtract,
                )
                msk2[t] = m2

            for t in range(NPRE):
                build_mask2(t)

            # A exps packed [A1; A2] for the final combine (off critical path)
            aio = cpool.tile([128, NS], F32)
            nc.scalar.activation(
                aio[0:64, :], f1own_bc128[0:64, :], AF.Exp, bias=bias_bc[0:64, 64:65]
            )
            nc.scalar.activation(
                aio[64:128, :], f1own_bc128[64:128, :], AF.Exp, scale=0.01,
                bias=css[64:128, 0:1]
            )

            # ---- transpose m rows -> columns via PE, finalize scB (all j) ----
            magg = cpool.tile([16, NS], BF16)
            nc.sync.dma_start(magg[:], ag_out[:])
            trall = psT.tile([128, 8, 8, 2], BF16, tag="tr", bufs=1)
            for b in range(8):
                nc.tensor.transpose(
                    trall[:, b, :, :], magg[:, b * 128:(b + 1) * 128], ident16[:]
                )
            m1v = trall[:, :, :, 0].rearrange("p b k -> p k b")
            m2v = trall[:, :, :, 1].rearrange("p b k -> p k b")

            # D = B1(m1 + SA1/2) + B2(SA2/2 - m2) ; scBk = Bk/D
            u1 = rpool.tile([128, NT], F32)
            nc.vector.tensor_scalar(u1[:], m1v, sah_bc[:, 0:1], None, OP.add)
            nc.vector.tensor_tensor(u1[:], u1[:], b1cols[:], OP.mult)
            u2 = rpool.tile([128, NT], F32)
            nc.vector.tensor_scalar(u2[:], m2v, -1.0, sah_bc[:, 1:2], OP.mult, OP.add)
            nc.vector.tensor_tensor(u2[:], u2[:], b2cols[:], OP.mult)
            nc.vector.tensor_tensor(u1[:], u1[:], u2[:], OP.add)
            rd = rpool.tile([128, NT], F32)
            nc.vector.reciprocal(rd[:], u1[:])
            scb1c = cpool.tile([128, NT], F32)
            nc.vector.tensor_tensor(scb1c[:], b1cols[:], rd[:], OP.mult)
            scb2c = cpool.tile([128, NT], F32)
            nc.vector.tensor_tensor(scb2c[:], b2cols[:], rd[:], OP.mult)
            scb1m = cpool.tile([128, 8, 8], BF16)
            nc.vector.tensor_copy(scb1m[:].rearrange("p a b -> p (a b)"), scb1c[:])
            scb2m = cpool.tile([128, 8, 8], BF16)
            nc.vector.tensor_copy(scb2m[:].rearrange("p a b -> p (a b)"), scb2c[:])

            # q columns per quarter (16 tiles): qa[qq][:, t%16] = [q1 | q2] of tile t
            qas = []
            for qq in range(4):
                qa = cpool.tile([128, 16, 2, 64], BF16, name=f"qa{qq}")
                for s, scbm in ((0, scb1m), (1, scb2m)):
                    sc = scbm[:, 2 * qq:2 * qq + 2, :].rearrange("p a b -> p (a b)")
                    nc.vector.tensor_tensor(
                        qa[:, :, s, :],
                        sfall[:, 16 * qq:16 * qq + 16, :],
                        sc.unsqueeze(2).broadcast_to([128, 16, 64]),
                        OP.mult,
                    )
                qas.append(qa)

            # ---- pass 2: ret^T accumulation over all j ----
            psum_ret = psM.tile([128, NS], F32, tag="ret")
            psum_t = psT.tile([128, 1], F32, tag="tcol", bufs=1)
            for t in range(NT):
                q12 = qas[t // 16][:, t % 16, :, :]
                if t + NPRE < NT:
                    build_mask2(t + NPRE)
                msk = msk2.pop(t)
                for h in range(2):
                    nc.tensor.matmul(
                        psum_ret[:, h * 512:(h + 1) * 512],
                        q12, msk[:, h * 512:(h + 1) * 512],
                        start=(t == 0), stop=(t == NT - 1),
                    )
                nc.tensor.matmul(
                    psum_t[:], q12, halfs_col[:],
                    start=(t == 0), stop=(t == NT - 1),
                )

            # ---- combine + elu (packed 128-partition tail) ----
            # tfix = [+T1/2 (rows 0:64); -T2/2 (rows 64:128)]
            tbig = cpool.tile([128, 1], F32)
            nc.vector.tensor_copy(tbig[0:64, :], psum_t[0:64, :])
            nc.vector.tensor_scalar(
                tbig[64:128, :], psum_t[64:128, :], -1.0, None, OP.mult
            )
            # rbigF = [R1 ; R2 - T2] = psum_ret + tfix, then prod = [A1 R1; A2(R2-T2)]
            rbigF = cpool.tile([128, NS], F32)
            nc.vector.tensor_scalar(
                rbigF[64:128, :], psum_ret[64:128, :], tbig[64:128, 0:1], None, OP.add
            )
            nc.vector.tensor_tensor(
                rbigF[64:128, :], rbigF[64:128, :], aio[64:128, :], OP.mult
            )
            p2s = cpool.tile([64, NS], F32)
            nc.sync.dma_start(p2s[:], rbigF[64:128, :])
            nc.vector.tensor_scalar(
                rbigF[0:64, :], psum_ret[0:64, :], tbig[0:64, 0:1], None, OP.add
            )
            nc.vector.tensor_tensor(
                rbigF[0:64, :], rbigF[0:64, :], aio[0:64, :], OP.mult
            )
            ret = cpool.tile([64, NS], F32)
            nc.vector.tensor_tensor(ret[:], rbigF[0:64, :], p2s[:], OP.subtract)
            # elu(x) = max(x,0) + min(exp(x)-1, 0)
            ex = cpool.tile([64, NS], F32)
            nc.scalar.activation(ex[:], ret[:], AF.Exp)
            nc.vector.tensor_scalar(ret[:], ret[:], 0.0, None, OP.max)
            nc.vector.tensor_scalar(ex[:], ex[:], -1.0, 0.0, OP.add, OP.min)
            nc.vector.tensor_tensor(ret[:], ret[:], ex[:], OP.add)
            nc.sync.dma_start(out[:], ret[:])

    nc.compile()
    return nc


@functools.lru_cache(maxsize=1)
def _get_nc():
    return build_kernel()


def make_in_maps(x, W1, b1, a1, ba1, a2, ba2, **kw):
    import ml_dtypes

    x = np.asarray(x, np.float32)
    W1 = np.asarray(W1, np.float32)
    b1 = np.asarray(b1, np.float32)
    a1 = np.asarray(a1, np.float32)
    a2 = np.asarray(a2, np.float32)
    ba1 = np.asarray(ba1, np.float32)
    ba2 = np.asarray(ba2, np.float32)

    bf = ml_dtypes.bfloat16
    w1t = W1.T                                            # [C, O]
    mov66 = np.concatenate(
        [w1t, (w1t @ a1)[:, None], (w1t @ a2)[:, None]], axis=1
    ).astype(bf)                                          # [C, 66]
    bias66 = np.concatenate(
        [b1, [b1 @ a1 + ba1[0]], [b1 @ a2 + ba2[0]]]
    ).astype(np.float32)[None, :]                         # [1, 66]
    xTf = np.ascontiguousarray(x[0].T).astype(bf)         # [C, N]

    in_maps = []
    for k in range(NCORES):
        in_maps.append({
            "xT": xTf,
            "xoT": np.ascontiguousarray(xTf[:, k * NS:(k + 1) * NS]),
            "mov66": mov66,
            "bias66": bias66,
        })
    return in_maps


def kernel(x, W1, b1, a1, ba1, a2, ba2, **kw):
    in_maps = make_in_maps(x, W1, b1, a1, ba1, a2, ba2)
    res = run_bass_kernel_spmd(_get_nc(), in_maps, core_ids=list(range(NCORES)))
    outp = np.empty((1, N, O), np.float32)
    for k in range(NCORES):
        outp[0, k * NS:(k + 1) * NS, :] = res.results[k]["out"].T
    return outp
